# revision 1
# baseline (speedup 1.0000x reference)
"""Trainium2 Bass kernel for nn_CustomConv2d: 3x3 conv, B=16, Cin=Cout=128, H=W=64.

Strategy:
  - Data-parallel over batch: 8 NeuronCores x 2 images each; the (128,128,9)
    weight is replicated (host pre-transposes it to [cin, k, cout] so tap k is
    a contiguous [cin, cout] stationary-operand slice).
  - Per image the feature map lives in SBUF as a 66x66 zero-padded plane
    (host-prepadded, so every DMA is fully contiguous):
      row r in [-1,64], col c in [-1,64] at offset (r+1)*66 + (c+1).
  - Conv = 9 accumulating PE matmuls per 8-row output block (contraction over
    Cin=128 on the partition dim).  Tap (dy,dx) reads the 2D window
    [[66,8],[1,64]] at offset (y0+dy)*66 + dx; the padding zeros make every
    tap exact, so there is no edge fixup of any kind.
  - Matmuls run in float32r (TF32-like: 1+8+11 bits, 1 cycle/row vs 4 for
    fp32; the [8,64] window APs satisfy the fp32r even-count/alignment ISA
    rules).  Inputs are pre-rounded to fp32r on the host (RNE on the top 20
    bits), so DMA needs no cast and on-device numerics are deterministic.
  - The PE is pre-warmed with bf16 dummy matmuls while the first DMAs stream
    (HAM un-throttles the PE clock 1.2->2.4 GHz only after ~3.4us of array
    activity), and the input is chunked so block 0 can start as soon as the
    first 12 padded rows + 6 weight taps have landed.
"""

import numpy as np

import concourse.bass as bass  # noqa: F401  (registers bass types)
import concourse.tile as tile
import concourse.mybir as mybir
from concourse import bacc, bass_utils

F32 = mybir.dt.float32
F32R = mybir.dt.float32r
BF16 = mybir.dt.bfloat16

B, CIN, COUT, KK, H, W = 16, 128, 128, 3, 64, 64
NCORES = 8
BPC = B // NCORES  # batches per core
HW = H * W         # 4096
PW = W + 2         # padded row length (66)
PH = H + 2         # padded rows (66)
XLEN = PH * PW     # 4356
ROWBLK = 8         # output rows per PSUM block (8*64=512 = one fp32 PSUM bank)
NBLK = H // ROWBLK

USE_F32R = True    # float32r matmuls (4x faster PE than fp32)
TRACE = False      # set True to capture an NTFF profile (fills LAST_EXEC_NS)
LAST_EXEC_NS = None

_CACHE = {}


def _build():
    mmdt = F32R if USE_F32R else F32
    nc = bacc.Bacc("TRN2", target_bir_lowering=False, debug=False, num_devices=NCORES)
    x_d = nc.dram_tensor("x", [BPC, CIN, XLEN], mmdt, kind="ExternalInput").ap()
    w_d = nc.dram_tensor("w", [CIN, KK * KK * COUT], mmdt, kind="ExternalInput").ap()
    o_d = nc.dram_tensor("o", [BPC, COUT, HW], F32, kind="ExternalOutput").ap()

    with tile.TileContext(nc) as tc:
        with (
            tc.tile_pool(name="wt", bufs=1) as wtp,
            tc.tile_pool(name="xin", bufs=2) as xp,
            tc.tile_pool(name="ps", bufs=4, space="PSUM") as pp,
            tc.tile_pool(name="ot", bufs=4) as op,
            tc.tile_pool(name="warm", bufs=1) as wmp,
            tc.tile_pool(name="warmps", bufs=1, space="PSUM") as wpp,
        ):
            # PE warm-up: HAM releases the clock gate only after ~3.4us of
            # sustained array activity; bf16 N=512 matmuls on a zeroed tile
            # keep the array busy while the first input DMAs stream.
            wz = wmp.tile([CIN, 4 * COUT], BF16)
            nc.gpsimd.memset(wz[:], 0.0)
            wps = wpp.tile([COUT, 4 * COUT], F32)
            for _ in range(8):
                nc.tensor.matmul(wps[:], wz[:, :COUT], wz[:], start=True, stop=True)

            # Latency-critical loads on the SP HWDGE ring, in the order the
            # PE consumes them.  Block yb touches padded rows [8*yb, 8*yb+9].
            wt = wtp.tile([CIN, KK * KK * COUT], mmdt)
            CHUNKS = [(0, 10), (10, 24), (24, 42), (42, 60), (60, PH)]

            xins = []
            for lb in range(BPC):
                xin = xp.tile([CIN, XLEN], mmdt, tag="xin")
                xins.append(xin)
            nc.sync.dma_start(wt[:, : 3 * COUT], w_d[:, : 3 * COUT])
            r0, r1 = CHUNKS[0]
            nc.sync.dma_start(
                xins[0][:, PW * r0 : PW * r1], x_d[0][:, PW * r0 : PW * r1]
            )
            nc.sync.dma_start(wt[:, 3 * COUT : 6 * COUT], w_d[:, 3 * COUT : 6 * COUT])
            nc.sync.dma_start(wt[:, 6 * COUT :], w_d[:, 6 * COUT :])
            for lb in range(BPC):
                for r0, r1 in CHUNKS[1:] if lb == 0 else CHUNKS:
                    nc.sync.dma_start(
                        xins[lb][:, PW * r0 : PW * r1], x_d[lb][:, PW * r0 : PW * r1]
                    )

            for lb in range(BPC):
                xrf = xins[lb][:].rearrange("p (r c) -> p r c", c=PW)  # [128,66,66]
                for yb in range(NBLK):
                    y0 = yb * ROWBLK
                    ps = pp.tile([COUT, ROWBLK * W], F32)
                    first = True
                    for dy in range(KK):
                        for dx in range(KK):
                            nc.tensor.matmul(
                                ps[:],
                                wt[:, (dy * KK + dx) * COUT : (dy * KK + dx + 1) * COUT],
                                xrf[:, y0 + dy : y0 + dy + ROWBLK, dx : dx + W],
                                start=first,
                                stop=(dy == KK - 1 and dx == KK - 1),
                            )
                            first = False
                    ot = op.tile([COUT, ROWBLK * W], F32)
                    if lb == BPC - 1 and yb == NBLK - 1:
                        # final block in halves so copy/store pipeline and the
                        # kernel-exit drain starts sooner
                        hw2 = ROWBLK * W // 2
                        for h_, eng in ((0, nc.scalar), (1, nc.sync)):
                            sl = slice(h_ * hw2, (h_ + 1) * hw2)
                            nc.vector.tensor_copy(ot[:, sl], ps[:, sl])
                            eng.dma_start(
                                o_d[lb][:, W * y0 + h_ * hw2 : W * y0 + (h_ + 1) * hw2],
                                ot[:, sl],
                            )
                    else:
                        nc.vector.tensor_copy(ot[:], ps[:])
                        nc.scalar.dma_start(
                            o_d[lb][:, W * y0 : W * y0 + ROWBLK * W], ot[:]
                        )
    nc.compile()
    return nc


def _get_nc():
    key = ("nc", USE_F32R)
    if key not in _CACHE:
        _CACHE[key] = _build()
    return _CACHE[key]


def _round_f32r(a):
    """RNE-round fp32 values to fp32r (keep top 20 bits: 1s+8e+11m)."""
    u = np.ascontiguousarray(a, dtype=np.float32).view(np.uint32)
    lsb = (u >> np.uint32(12)) & np.uint32(1)
    r = u + np.uint32(0x7FF) + lsb
    return (r & np.uint32(0xFFFFF000)).view(np.float32)


def kernel(x, weights):
    """x: [16,128,64,64] f32; weights: [128,128,9] f32 -> [2048,64,64] f32."""
    global LAST_EXEC_NS
    x = np.asarray(x, dtype=np.float32)
    w = np.asarray(weights, dtype=np.float32)
    # [cout, cin, k] -> [cin, k, cout] so tap k is a contiguous lhsT slice
    wT = np.ascontiguousarray(w.transpose(1, 2, 0)).reshape(CIN, KK * KK * COUT)
    xpad = np.zeros((B, CIN, PH, PW), np.float32)
    xpad[:, :, 1 : H + 1, 1 : W + 1] = x
    xpad = xpad.reshape(B, CIN, XLEN)
    if USE_F32R:
        wT = _round_f32r(wT)
        xpad = _round_f32r(xpad.reshape(-1)).reshape(B, CIN, XLEN)

    nc = _get_nc()
    xr = xpad.reshape(NCORES, BPC, CIN, XLEN)
    in_maps = [{"x": np.ascontiguousarray(xr[c]), "w": wT} for c in range(NCORES)]

    res = bass_utils.run_bass_kernel_spmd(
        nc, in_maps, core_ids=list(range(NCORES)), trace=TRACE
    )
    LAST_EXEC_NS = res.exec_time_ns

    arr = np.stack([res.results[c]["o"] for c in range(NCORES)])  # [8, 2, 128, 4096]
    # out[cout*B + b] = conv[b, cout], with b = core*BPC + lb
    arr = arr.transpose(2, 0, 1, 3).reshape(COUT, B, H, W)
    return np.ascontiguousarray(arr.reshape(COUT * B, H, W))

